# revision 10
# baseline (speedup 1.0000x reference)
"""BertScore model kernel for Trainium2 (8 NeuronCores, SPMD data-parallel over B).

Reference: cosine-normalized per-layer token reps, per-(layer,batch) similarity
matrix dots = h1 @ h2^T (256x256, contraction D=1024), ragged masked max over
rows/cols + masked means -> s1,s2, F1 harmonic mean -> (B,NL) features,
BatchNorm over batch, linear head -> (B,).

Device strategy (per core, 8 batches):
- Inputs quantized host-side to fp8 e4m3 (x16 scale so values sit in the
  normal range; dots come out x256, host divides). fp8 halves HBM traffic vs
  fp16 (16.8 MB/core, ~47 us at the ~358 GB/s per-core HBM share) and
  enables DoubleRow matmuls (2 contraction rows per partition per pass).
- DRAM layout is partition-major, packed in exact consumption order:
  hp[p, l, b, kt, e, i] = h[l, b, d, i] with d = kt*256 + e*128 + p, so every
  DMA tile is one contiguous run per partition (8 KB packets at u=4; the
  273 GB/s fp16 baseline was descriptor-rate-limited at 4 KB packets).
- All input DMAs are issued upfront (first chunks small so compute starts
  ~11 us in); SBUF holds the full working set, so the 16 DMA engines stream
  FIFO flat-out and compute chases tile completion.
- Per batch: 8 DoubleRow matmuls (4 kt x 2 i-halves, K=256) accumulate
  scaled dots into one PSUM bank [128, 2, 256]; a 9th DoubleRow matmul
  (lhsT=2.0s, rhs=m2 as 0/-240 fp8) adds -960 to invalid j columns (fp32r
  mask matmuls measured 5x slower, ~630 ns of PE stall per batch); ACT
  copies PSUM->SBUF fp16 adding the per-partition row mask m1 (0/-30000);
  one DVE max-reduce (fp16 in+out, 2x mode) over j gives both RM halves;
  4 PE fp16 transposes (~60 ns issue each) + one DVE max-reduce give CM.
  RM/CM accumulate per-layer fp16 and flush per-layer.
- Host epilogue: unscale maxes, ragged means, F1, BatchNorm over the full
  batch (the cross-core reduction), linear head. Invalid rows/cols are
  dropped by the host masks, so their (very negative) values never matter.
"""
import os
import numpy as np

NL, B, L1, L2, D = 4, 64, 256, 256, 1024
NCORES = 8
BB = B // NCORES          # batches per core
KT = 4                    # DoubleRow contraction tiles (K=256 each)
NEG = -30000.0            # m1 additive row mask (fp16-safe)
M2V = -240.0              # m2 fp8 mask value; column offset = 4*M2V = -960
SCALE = 16.0              # input quantization scale; dots scale = SCALE**2
BN_EPS = 1e-8
LOGIT_SCALE = 1.0

PSB = int(os.environ.get("BSM_PSB", "3"))        # PSUM dps bufs
DSB = int(os.environ.get("BSM_DSB", "6"))        # dsb sbuf bufs
PTB = int(os.environ.get("BSM_PTB", "3"))        # PSUM dT bufs
# batch counts per DMA chunk, per layer (l=0 ramps up so compute starts early)
CHUNKS0 = [int(x) for x in os.environ.get("BSM_CHUNKS0", "1,1,2,4").split(",")]
CHUNKSN = [int(x) for x in os.environ.get("BSM_CHUNKSN", "4,4").split(",")]

_CACHE = {}


def _chunks(l):
    return CHUNKS0 if l == 0 else CHUNKSN


def _build(psb, dsb_bufs, ptb, chunks0, chunksn):
    import concourse.bacc as bacc
    import concourse.bass as bass
    import concourse.mybir as mybir
    import concourse.tile as tile
    from concourse.masks import make_identity

    f32 = mybir.dt.float32
    f16 = mybir.dt.float16
    f8 = mybir.dt.float8e4

    BSTR = KT * 2 * L1                    # per-batch per-partition elements
    ntiles = NL * len(chunksn) - len(chunksn) + len(chunks0)

    nc = bacc.Bacc("TRN2", target_bir_lowering=False, debug=False,
                   num_devices=NCORES)

    h1d = nc.dram_tensor("h1d", [128, NL * BB * BSTR], f8, kind="ExternalInput")
    h2d = nc.dram_tensor("h2d", [128, NL * BB * BSTR], f8, kind="ExternalInput")
    # m1 as per-partition bias columns: m1c[p,b,half] = m1[b, half*128+p]
    m1c = nc.dram_tensor("m1c", [128, BB, 2], f32, kind="ExternalInput")
    # m2 as fp8 0/-240, laid out [b, e, (it j)] (same row for e=0,1 and it=0,1)
    m2d = nc.dram_tensor("m2", [BB, 2, 2 * L2], f8, kind="ExternalInput")
    twosd = nc.dram_tensor("twos", [1, 2, 128], f8, kind="ExternalInput")
    NCOL = NL * BB * 2
    rmd = nc.dram_tensor("rm", [128, NCOL], f16, kind="ExternalOutput")
    cmd = nc.dram_tensor("cm", [128, NCOL], f16, kind="ExternalOutput")

    DR = mybir.MatmulPerfMode.DoubleRow
    vmax = mybir.AluOpType.max
    X = mybir.AxisListType.X
    IDENT = mybir.ActivationFunctionType.Identity

    with tile.TileContext(nc) as tc:
        with tc.tile_pool(name="consts", bufs=1) as consts, \
             tc.tile_pool(name="io", bufs=ntiles) as io, \
             tc.tile_pool(name="dsbp", bufs=dsb_bufs) as dsbp, \
             tc.tile_pool(name="accp", bufs=1) as accp, \
             tc.tile_pool(name="ps", bufs=psb, space="PSUM") as ps, \
             tc.tile_pool(name="psT", bufs=ptb, space="PSUM") as psT:

            h1ap = h1d.ap()
            h2ap = h2d.ap()

            # first chunk pair goes out before anything else
            blks = []  # (h1blk, h2blk, nb) per chunk, consumption order
            order = [(l, ci) for l in range(NL)
                     for ci in range(len(_chunks_local(l, chunks0, chunksn)))]

            def issue(l, ci):
                chs = _chunks_local(l, chunks0, chunksn)
                u = chs[ci]
                b0 = l * BB + sum(chs[:ci])
                off = b0 * BSTR
                h1blk = io.tile([128, u * BSTR], f8, tag="h1", name=f"h1_{l}_{ci}")
                nc.sync.dma_start(out=h1blk, in_=h1ap[:, off:off + u * BSTR])
                h2blk = io.tile([128, u * BSTR], f8, tag="h2", name=f"h2_{l}_{ci}")
                nc.sync.dma_start(out=h2blk, in_=h2ap[:, off:off + u * BSTR])
                blks.append((h1blk, h2blk, u))

            issue(0, 0)

            ident = consts.tile([128, 128], f16)
            make_identity(nc, ident)
            twos = consts.tile([1, 2, 128], f8)
            nc.sync.dma_start(out=twos, in_=twosd.ap())
            m2sb = consts.tile([1, BB, 2, 2 * L2], f8)
            m2ap = m2d.ap()
            nc.sync.dma_start(out=m2sb, in_=bass.AP(
                tensor=m2ap.tensor, offset=m2ap.offset,
                ap=[[0, 1], [4 * L2, BB], [2 * L2, 2], [1, 2 * L2]]))
            m1sb = consts.tile([128, BB, 2], f32)
            nc.sync.dma_start(out=m1sb, in_=m1c.ap())

            for l, ci in order[1:]:
                issue(l, ci)

            RMs, CMs = [], []
            for l in range(NL):
                rmt = accp.tile([128, BB * 2], f16, tag=f"rm{l}")
                cmt = accp.tile([128, BB * 2], f16, tag=f"cm{l}")
                RMs.append(rmt)
                CMs.append(cmt)

            def transpose_cm(dsb, l, b):
                # transpose -> dT[p=j%128, jt, it*128+q=i], then col max
                dT = psT.tile([128, 2, L1], f16, tag="dT", name=f"dT{l}_{b}")
                for jt in range(2):
                    for it in range(2):
                        nc.tensor.transpose(
                            out=dT[:, jt, it * 128:(it + 1) * 128],
                            in_=dsb[:, it, jt * 128:(jt + 1) * 128],
                            identity=ident)
                nc.vector.tensor_reduce(
                    out=CMs[l][:, b * 2:b * 2 + 2], in_=dT,
                    axis=X, op=vmax)
                if b == BB - 1:
                    c0 = l * BB * 2
                    nc.sync.dma_start(out=cmd.ap()[:, c0:c0 + BB * 2],
                                      in_=CMs[l])

            pending = None  # (dsb, l, b) whose transpose+CM is deferred
            bi = 0
            for l in range(NL):
                for ci in range(len(_chunks_local(l, chunks0, chunksn))):
                    h1blk, h2blk, u = blks[bi]
                    bi += 1
                    h1v = h1blk.rearrange("p (b k e i) -> p b k e i",
                                          b=u, k=KT, e=2)
                    h2v = h2blk.rearrange("p (b k e j) -> p b k e j",
                                          b=u, k=KT, e=2)
                    chs = _chunks_local(l, chunks0, chunksn)
                    for ub in range(u):
                        b = sum(chs[:ci]) + ub
                        # scaled dots, both i-halves: [128, 2(it), 256(j)]
                        dps = ps.tile([128, 2, L2], f32, tag="dots")
                        for it in range(2):
                            for k in range(KT):
                                # start only on the group's first matmul: its
                                # pending-zero covers the whole 2KB bank
                                # (both it halves); a second start would wipe
                                # the it=0 half
                                nc.tensor.matmul(
                                    out=dps[:, it, :],
                                    lhsT=h1v[:, ub, k, :,
                                             it * 128:(it + 1) * 128],
                                    rhs=h2v[:, ub, k, :, :],
                                    start=(it == 0 and k == 0), stop=False,
                                    perf_mode=DR)
                        # -960 on invalid j columns of both halves (fp8 DR:
                        # 2 k-pairs x 2.0 x -240)
                        nc.tensor.matmul(out=dps, lhsT=twos,
                                         rhs=m2sb[:, b, :, :],
                                         start=False, stop=True,
                                         perf_mode=DR)
                        # PSUM->SBUF fp16 with per-partition m1[i] bias
                        dsb = dsbp.tile([128, 2, L2], f16, tag="dsb")
                        for it in range(2):
                            nc.scalar.activation(
                                out=dsb[:, it, :], in_=dps[:, it, :],
                                func=IDENT, bias=m1sb[:, b, it:it + 1])
                        # row max over j for both halves
                        nc.vector.tensor_reduce(
                            out=RMs[l][:, b * 2:b * 2 + 2], in_=dsb,
                            axis=X, op=vmax)
                        if b == BB - 1:
                            c0 = l * BB * 2
                            nc.sync.dma_start(
                                out=rmd.ap()[:, c0:c0 + BB * 2], in_=RMs[l])
                        # transposes+CM run one batch behind so they never
                        # head-of-line-block the next batch's matmuls (they
                        # wait on this batch's ACT otherwise)
                        if pending is not None:
                            transpose_cm(*pending)
                        pending = (dsb, l, b)
            transpose_cm(*pending)

    nc.finalize()
    return nc


def _chunks_local(l, chunks0, chunksn):
    return chunks0 if l == 0 else chunksn


def _get_nc():
    key = (PSB, DSB, PTB, tuple(CHUNKS0), tuple(CHUNKSN))
    if key not in _CACHE:
        _CACHE[key] = _build(*key)
    return _CACHE[key]


def _host_prep(reps1, reps2, len1, len2):
    """Normalize, scale, fp8-quantize, pack partition-major; per-core maps."""
    import ml_dtypes
    f8 = ml_dtypes.float8_e4m3

    def prep(r):
        r = np.asarray(r, dtype=np.float32)
        n = np.sqrt(np.einsum('lbid,lbid->lbi', r, r))
        h = (r * (SCALE / n[..., None])).astype(f8)     # (NL, B, L, D)
        # d = kt*256 + e*128 + p  ->  hp[p, l, b, kt, e, i]
        hp = h.transpose(0, 1, 3, 2).reshape(NL, B, KT, 2, 128, L1)
        hp = np.ascontiguousarray(hp.transpose(4, 0, 1, 2, 3, 5))
        return hp.reshape(128, NL * B * KT * 2 * L1)

    h1p = prep(reps1)
    h2p = prep(reps2)
    len1 = np.asarray(len1).astype(np.int64)
    len2 = np.asarray(len2).astype(np.int64)
    ar1 = np.arange(L1)[None, :]
    ar2 = np.arange(L2)[None, :]
    m1 = np.where(ar1 < len1[:, None], 0.0, NEG).astype(np.float32)  # (B, L1)
    m2 = np.where(ar2 < len2[:, None], 0.0, M2V).astype(np.float32)
    # (B, L1) -> (B, 2, 128) -> (128, B, 2)
    m1c = np.ascontiguousarray(m1.reshape(B, 2, 128).transpose(2, 0, 1))
    # m2 fp8 [b, e, (it j)]: same m2 row replicated 2(e) x 2(it)
    m2q = np.ascontiguousarray(np.broadcast_to(
        np.tile(m2, (1, 2))[:, None, :], (B, 2, 2 * L2))).astype(f8)

    BSTR = KT * 2 * L1
    in_maps = []
    for c in range(NCORES):
        sl = slice(c * BB, (c + 1) * BB)
        in_maps.append({
            "h1d": np.ascontiguousarray(
                h1p.reshape(128, NL, B, BSTR)[:, :, sl]).reshape(128, -1),
            "h2d": np.ascontiguousarray(
                h2p.reshape(128, NL, B, BSTR)[:, :, sl]).reshape(128, -1),
            "m1c": np.ascontiguousarray(m1c[:, sl]),
            "m2": np.ascontiguousarray(m2q[sl]),
            "twos": np.full((1, 2, 128), 2.0, dtype=f8),
        })
    return in_maps, len1, len2


def _epilogue(results, len1, len2, w, b):
    """rm/cm (128, NL*BB*2) per core -> s1,s2 -> F1 -> BatchNorm -> head."""
    inv = 1.0 / (SCALE * SCALE)
    maxv_rows = np.empty((NL, B, L1), dtype=np.float64)  # max over valid j
    maxv_cols = np.empty((NL, B, L2), dtype=np.float64)  # max over valid i
    for c, res in enumerate(results):
        rm = np.asarray(res["rm"]).astype(np.float64) * inv  # (128, NCOL)
        cm = np.asarray(res["cm"]).astype(np.float64) * inv
        # col = (l*BB + b)*2 + half ; partition p -> index half*128 + p
        rm_r = rm.T.reshape(NL, BB, 2, 128).reshape(NL, BB, 256)
        cm_r = cm.T.reshape(NL, BB, 2, 128).reshape(NL, BB, 256)
        maxv_rows[:, c * BB:(c + 1) * BB] = rm_r
        maxv_cols[:, c * BB:(c + 1) * BB] = cm_r

    ar1 = np.arange(L1)[None, :]
    ar2 = np.arange(L2)[None, :]
    mask1 = (ar1 < len1[:, None])  # (B, L1)
    mask2 = (ar2 < len2[:, None])
    n1 = len1.astype(np.float64)
    n2 = len2.astype(np.float64)

    s2 = np.where(mask1[None], maxv_rows, 0.0).sum(axis=2) / n1[None]  # (NL,B)
    s1 = np.where(mask2[None], maxv_cols, 0.0).sum(axis=2) / n2[None]
    feat = (2.0 * s1 * s2 / (s1 + s2)).T                    # (B, NL)
    mean = feat.mean(axis=0, keepdims=True)
    var = ((feat - mean) ** 2).mean(axis=0, keepdims=True)
    feat = (feat - mean) / np.sqrt(var + BN_EPS)
    w = np.asarray(w, dtype=np.float64)
    bb = np.asarray(b, dtype=np.float64)
    out = LOGIT_SCALE * (feat @ w.T + bb)[:, 0]
    return out.astype(np.float32)


LAST_RUN = {}


def kernel(reps1, reps2, len1, len2, w, b):
    from concourse.bass_utils import run_bass_kernel_spmd

    nc = _get_nc()
    in_maps, l1, l2 = _host_prep(reps1, reps2, len1, len2)
    res = run_bass_kernel_spmd(nc, in_maps, list(range(NCORES)))
    LAST_RUN["results"] = res
    LAST_RUN["in_maps"] = in_maps
    return _epilogue(res.results, l1, l2, w, b)
